# revision 27
# baseline (speedup 1.0000x reference)
"""Trainium2 Bass kernel for nn_BasicBlock_1w4a_LUT (binarized 3x3 conv + LUT bucketize).

Data-parallel over batch: 8 NeuronCores x 4 images each; no cross-core
communication. Full inputs in, full output out; shard/unshard on the host.

v3: 4 concurrent PE streams via 64x64 array tiling (T0/T2/T8/T10), fp16
hi-only activations (K=64), 2-bank PSUM tiles, batched DVE.

Host prep:
  - Binarize the weights exactly as the reference does; the pow2 scale sw is
    folded into the LUT thresholds so device weights are exactly +/-1.
  - x is rounded to fp16 ("hi"). The dropped residual perturbs the conv
    output by sigma ~ 5e-3 against threshold spacing ~1, measured rel err
    ~5e-3 on the bucketized output (gate is 2e-2). PE multiplies fp16
    exactly and accumulates fp32.
  - Each 112-row image is split into 4 bands of 28 rows, one per PE tile.
    Bands A,B live in SBUF partitions 0-63 (padded rows 0..57), bands C,D
    in partitions 64-127 (padded rows 56..113); 64 channels per partition
    group, 2-row halo duplicated.
  - Per-channel affine z = y*s + b chosen so bucketize thresholds map
    tau3 -> 0 and tau5 -> 1 (frees DVE scalar slots).

Device, per image:
  - 7 quad-steps of 4-row chunks: per step, each of the 4 64x64 PE tiles
    accumulates 9 taps x 448 columns into its PSUM half-bank; the 4 streams
    run concurrently (1 col/cycle/tile at 2.4 GHz). PSUM tiles span 2 banks
    ([128, 2, 448] padded to 512) so one chunk-pair evacuates per scalar op.
  - ScalarE applies the per-channel affine out of PSUM; two custom DVE ops
    compute out = sum_k [z > tau_k] over all 7 thresholds in 2 passes over
    [128, 1792/1344] batches, writing u8.
  - PE warm-up matmuls run on the weights tile itself (no memset
    dependency) so the HAM clock gate opens (1.2 -> 2.4 GHz) while the
    first input slab lands.
"""

import numpy as np

# ---- problem constants (hardcoded per contract) ----
B, Cin, Cout, H, W = 32, 64, 64, 112, 112
NCORES = 8
BPC = B // NCORES          # images per core
HP = H + 2                 # 114 padded rows
WPAD = H + 2               # 114 padded cols
BAND = 28                  # rows per band; 4 bands per image
NB = 7                     # 4-row chunks per band
NCH = 4 * W                # 448 pixels per chunk (one PSUM bank)
SLABR = 58                 # padded rows per slab half (halo of 2 shared)
SLABF = SLABR * WPAD       # 6612 fp16 per partition per image
HALFPX = BAND * W          # 3136 pixels per band
OUTF = 2 * HALFPX          # 6272 output pixels per partition per image
NTAPS = 9
NWARM = 13                 # PE warm-up matmuls

_built = []
last_results = None


def _register_dve_ops():
    from concourse.dve_spec import (
        Spec, Src0, Src1, C0, C1, C3, One, Zero, lower,
        _spill_c3_to_src1, _has_src1,
    )
    import concourse.dve_ops as dve_ops
    from concourse.dve_ops import DveOp
    from concourse.dve_uop import DveOpSpec

    def register_op(name, spec):
        if name in dve_ops._SUB_OPCODE_FOR_NAME:
            for op in dve_ops.OPS:
                if op.name == name:
                    return op
            raise RuntimeError(name)
        row = max(dve_ops._SUB_OPCODE_FOR_NAME.values()) + 1
        assert row < 0x20, "custom-DVE opcode rows exhausted"
        shas = {}
        for ver in ("v3", "v4"):
            s = DveOpSpec(name=name, opcode=row, uops=lower(spec, ver=ver),
                          rd1_en=_has_src1(spec))
            shas[ver] = s.sha(ver)
        op = DveOp(name, spec, subdim=False, uops_sha=shas)
        dve_ops.OPS.append(op)
        dve_ops.CUSTOM_DVE_SPECS[name] = spec
        dve_ops._SUB_OPCODE_FOR_NAME[name] = row
        return op

    # u = (z>tau0) + (z>tau1) + (z>tau2);  tau2 rides C3 (spilled to in1 [P,1])
    bucket3 = register_op(
        "BUCKET3_ANT",
        Spec(
            body=_spill_c3_to_src1(((Src0 > C0) + (Src0 > C1)) + (Src0 > C3)),
            reference=lambda in0, in1, s0, s1, imm2: (
                (in0 > s0).astype(np.float32) + (in0 > s1)
                + (in0 > np.asarray(in1, np.float32).reshape(-1, 1))
            ),
        ),
    )
    # out = (z>0) + (z>1) + (z>tau4) + (z>tau6) + u
    bucket4acc = register_op(
        "BUCKET4ACC_ANT",
        Spec(
            body=(((Src0 > Zero) + (Src0 > One))
                  + ((Src0 > C0) + (Src0 > C1))) + Src1,
            reference=lambda in0, in1, s0, s1, imm2: (
                (in0 > 0).astype(np.float32) + (in0 > 1)
                + (in0 > s0) + (in0 > s1) + in1
            ),
        ),
    )
    return bucket3, bucket4acc


def _build():
    """Trace + compile the per-core Bass kernel (once per process)."""
    if _built:
        return _built[0]

    import concourse.bacc as bacc
    import concourse.mybir as mybir
    import concourse.tile as tile

    bucket3, bucket4acc = _register_dve_ops()

    f32, f16, u8 = mybir.dt.float32, mybir.dt.float16, mybir.dt.uint8
    nc = bacc.Bacc("TRN2", target_bir_lowering=False, debug=False,
                   num_devices=NCORES)

    xin_t = nc.dram_tensor("xin", [BPC, 128, SLABF], f16, kind="ExternalInput")
    wts_t = nc.dram_tensor("wts", [128, NTAPS, Cout], f16, kind="ExternalInput")
    nrm_t = nc.dram_tensor("nrm", [128, 7], f32, kind="ExternalInput")
    out_t = nc.dram_tensor("out", [BPC, 128, OUTF], u8, kind="ExternalOutput")

    with tile.TileContext(nc) as tc:
        with (
            tc.tile_pool(name="const", bufs=1) as cpool,
            tc.tile_pool(name="slab", bufs=2) as spool,
            tc.tile_pool(name="psum", bufs=4, space="PSUM") as ppool,
            tc.tile_pool(name="z", bufs=4) as zpool,
            tc.tile_pool(name="u", bufs=4) as upool,
            tc.tile_pool(name="o", bufs=2) as opool,
        ):
            # Input slab pieces matched to consumption order: quad-group g
            # only needs its own band rows — A/C rows 4g..4g+9 (low cols)
            # and B/D rows 28+4g..28+4g+9 (high cols). Ranges in fp16 cols
            # (x114 per padded row), deduplicated across groups:
            ga = [(0, 1140), (1140, 2052), (2052, 2964), (2964, 3192)]
            gb = [(3192, 4332), (4332, 5244), (5244, 6156), (6156, SLABF)]
            # weights first (small) — they gate the first real matmuls; the
            # group-0 ranges are split across both HWDGE queues so they land
            # as early as possible
            slab0 = spool.tile([128, SLABF], f16, tag="slab")
            wts = cpool.tile([128, NTAPS, Cout], f16)
            nrm = cpool.tile([128, 7], f32)
            nc.scalar.dma_start(out=wts[:], in_=wts_t.ap())
            nc.sync.dma_start(out=slab0[:, 0:570],
                              in_=xin_t.ap()[0, :, 0:570])
            nc.scalar.dma_start(out=slab0[:, 570:1140],
                                in_=xin_t.ap()[0, :, 570:1140])
            nc.sync.dma_start(out=slab0[:, 3192:3762],
                              in_=xin_t.ap()[0, :, 3192:3762])
            nc.scalar.dma_start(out=slab0[:, 3762:4332],
                                in_=xin_t.ap()[0, :, 3762:4332])
            nc.scalar.dma_start(out=nrm[:], in_=nrm_t.ap())
            for n, (lo, hi) in enumerate(
                    (ga[1], gb[1], ga[2], gb[2], ga[3], gb[3])):
                eng = nc.sync if n % 2 == 0 else nc.scalar
                eng.dma_start(out=slab0[:, lo:hi], in_=xin_t.ap()[0, :, lo:hi])

            # PE warm-up: junk matmuls on a DVE-memset tile — no DMA
            # dependency, so they issue as soon as the engines come up and
            # the HAM clock gate opens (1.2 -> 2.4 GHz) before the real
            # matmuls start. Same 64x64 tiling mode as the real matmuls
            # (mode switches drain the PE); only the two column tiles of
            # row-group 0 (row tiles must not hit the same PSUM bank
            # concurrently).
            # warm-ups run serialized on the FULL 128x128 array (no tiling)
            # so the HAM sees full-occupancy sustained busy: NWARM x 373ns
            # > the 3.4us window. The one mode-switch drain into 64x64 when
            # the real matmuls arrive costs ~0.3us.
            wu = cpool.tile([128, NCH], f16)
            nc.vector.memset(wu[:], 0.0)
            wps = ppool.tile([128, 2, NCH], f32, name="ps", tag="ps",
                             padded_shape=[128, 2, 512])
            for i in range(NWARM):
                nc.tensor.matmul(wps[:, i % 2, :], wu[:, 0:128], wu[:],
                                 start=True, stop=True)

            scale, bias = nrm[:, 0:1], nrm[:, 1:2]
            tau0, tau1, tau2 = nrm[:, 2:3], nrm[:, 3:4], nrm[:, 4:5]
            tau4, tau6 = nrm[:, 5:6], nrm[:, 6:7]

            for b in range(BPC):
                # input slab pieces in consumption order (see head comment)
                if b == 0:
                    slab = slab0
                else:
                    slab = spool.tile([128, SLABF], f16, tag="slab")
                    pieces = [ga[0], gb[0], ga[1], gb[1],
                              ga[2], gb[2], ga[3], gb[3]]
                    for n, (lo, hi) in enumerate(pieces):
                        eng = nc.sync if n % 2 == 0 else nc.scalar
                        eng.dma_start(out=slab[:, lo:hi],
                                      in_=xin_t.ap()[b, :, lo:hi])
                slabv = slab[:].rearrange("p (r w) -> p r w", w=WPAD)

                oslab = opool.tile([128, OUTF], u8)
                # DVE batches (chunk groups -> columns): small batches for
                # image 0 so the DVE chain starts as early as possible; the
                # DVE never idles once started, so later images use maximal
                # batches to minimize per-op overhead.
                if b == 0:
                    batching = ((((0,),), 448), (((1,),), 448),
                                (((2, 3),), 896), (((4, 5), (6,)), 1344))
                elif b == BPC - 1:
                    batching = ((((0, 1), (2, 3)), 1792), (((4, 5),), 896),
                                (((6,),), 448))
                else:
                    batching = ((((0, 1), (2, 3)), 1792),
                                (((4, 5), (6,)), 1344))
                coff = 0
                for glist, bcols in batching:
                    batch_col = coff
                    coff += bcols
                    zA = zpool.tile([128, bcols], f32, name="zA", tag="z")
                    zB = zpool.tile([128, bcols], f32, name="zB", tag="z")
                    zoff = 0
                    for chunks in glist:
                        ncols = NCH * len(chunks)
                        psA = ppool.tile([128, 2, NCH], f32, name="ps",
                                         tag="ps", padded_shape=[128, 2, 512])
                        psB = ppool.tile([128, 2, NCH], f32, name="ps",
                                         tag="ps", padded_shape=[128, 2, 512])
                        # 4 concurrent streams: T0=(0,0) band A, T2=(0,64)
                        # band B, T8=(64,0) band C, T10=(64,64) band D.
                        for t in range(NTAPS):
                            dh, dw = divmod(t, 3)
                            st, sp = (t == 0), (t == NTAPS - 1)
                            for k, c in enumerate(chunks):
                                rlo = 4 * c + dh
                                rhi = BAND + rlo
                                nc.tensor.matmul(
                                    psA[0:64, k, :], wts[0:64, t, :],
                                    slabv[0:64, rlo:rlo + 4, dw:dw + W],
                                    tile_position=(0, 0), start=st, stop=sp)
                                nc.tensor.matmul(
                                    psA[64:128, k, :], wts[0:64, t, :],
                                    slabv[0:64, rhi:rhi + 4, dw:dw + W],
                                    tile_position=(0, 64), start=st, stop=sp)
                                nc.tensor.matmul(
                                    psB[0:64, k, :], wts[64:128, t, :],
                                    slabv[64:128, rlo:rlo + 4, dw:dw + W],
                                    tile_position=(64, 0), start=st, stop=sp)
                                nc.tensor.matmul(
                                    psB[64:128, k, :], wts[64:128, t, :],
                                    slabv[64:128, rhi:rhi + 4, dw:dw + W],
                                    tile_position=(64, 64), start=st, stop=sp)
                        # evacuate the 2-bank tile with one scalar op each
                        for z, ps in ((zA, psA), (zB, psB)):
                            nc.scalar.activation(
                                z[:, zoff:zoff + ncols].rearrange(
                                    "p (k c) -> p k c", c=NCH),
                                ps[:, 0:len(chunks), :],
                                mybir.ActivationFunctionType.Identity,
                                bias=bias, scale=scale)
                        zoff += ncols

                    # 2 DVE passes per batch per band-pair, u8 into oslab
                    for z, base in ((zA, 0), (zB, HALFPX)):
                        u = upool.tile([128, bcols], mybir.dt.bfloat16)
                        nc.vector._custom_dve(
                            bucket3, out=u[:], in0=z[:],
                            in1=tau2, s0=tau0, s1=tau1)
                        col = base + batch_col
                        nc.vector._custom_dve(
                            bucket4acc, out=oslab[:, col:col + bcols],
                            in0=z[:], in1=u[:], s0=tau4, s1=tau6)

                # split output DMA so early pieces leave while later chunks
                # are still being computed; finest for the last image (tail)
                if b == BPC - 1:
                    ocuts = [0, HALFPX, HALFPX + 1792, OUTF - 896,
                             OUTF - 448, OUTF]
                else:
                    ocuts = [0, HALFPX, OUTF]
                for lo, hi in zip(ocuts[:-1], ocuts[1:]):
                    nc.sync.dma_start(out=out_t.ap()[b, :, lo:hi],
                                      in_=oslab[:, lo:hi])

    nc.compile()
    _built.append(nc)
    return nc


def _binarize_weights(w):
    """Exactly the reference's fp32 binarization. Returns (sign in {-1,0,1}, sw)."""
    w = np.asarray(w, np.float32)
    C = w.shape[0]
    wf = w.reshape(C, -1)
    bw = w - wf.mean(-1)[:, None, None, None]
    bw = bw / bw.reshape(C, -1).std(-1, ddof=1)[:, None, None, None]
    mean_abs = np.abs(bw).reshape(C, -1).mean(-1)
    sw = np.exp2(np.round(np.log2(mean_abs))).astype(np.float32)
    return np.sign(bw).astype(np.float32), sw


def kernel(x, w, lut):
    x = np.ascontiguousarray(np.asarray(x, np.float32))
    w = np.asarray(w, np.float32)
    lut = np.asarray(lut, np.float32)

    nc = _build()
    from concourse import bass_utils

    # ---- weights: binarize + fold the pow2 scale into the thresholds ----
    sgn, sw = _binarize_weights(w)                     # sgn [Cout,Cin,3,3]
    t64 = lut.astype(np.float64) / sw[:, None]         # [Cout,7] thresholds

    # lhsT per tap: wts[ci, t, co] = sgn[co, ci, dh, dw]; rows 64-127 serve
    # the row-tiled PE tiles T8/T10 (same weights, SBUF partitions 64-127)
    wts = np.empty((128, NTAPS, Cout), np.float32)
    for t in range(NTAPS):
        wts[:Cin, t, :] = sgn[:, :, t // 3, t % 3].T
    wts[Cin:] = wts[:Cin]
    wts = wts.astype(np.float16)

    # ---- normalize params: z = y*s + b with tau3 -> 0, tau5 -> 1 ----
    # s>0 always; for degenerate channels (t5 == t3) use a huge power of two
    # so [z > 1] still decides [y > t3] exactly.
    t3, t5 = t64[:, 3], t64[:, 5]
    gap = t5 - t3
    s = np.where(gap > 0, 1.0 / np.where(gap > 0, gap, 1.0), 2.0 ** 100)
    bias = -t3 * s
    taus = (t64[:, [0, 1, 2, 4, 6]] - t3[:, None]) * s[:, None]
    half = np.stack([s, bias, taus[:, 0], taus[:, 1], taus[:, 2],
                     taus[:, 3], taus[:, 4]], axis=1).astype(np.float32)
    nrm = np.empty((128, 7), np.float32)
    nrm[:Cout] = half
    nrm[Cout:] = half

    # ---- fp16 slabs: bands A,B (padded rows 0..57) in partitions 0-63,
    # bands C,D (padded rows 56..113) in partitions 64-127 ----
    hi16 = x.astype(np.float16)
    xin = np.zeros((B, 128, SLABF), np.float16)
    view = xin.reshape(B, 128, SLABR, WPAD)
    view[:, :Cin, 1:58, 1:W + 1] = hi16[:, :, 0:57, :]
    view[:, Cin:, 0:57, 1:W + 1] = hi16[:, :, 55:112, :]

    # ---- run on the 8 cores (SPMD, batch-sharded) ----
    wts_np = np.ascontiguousarray(wts)
    nrm_np = np.ascontiguousarray(nrm)
    in_maps = [
        {
            "xin": np.ascontiguousarray(xin[c * BPC:(c + 1) * BPC]),
            "wts": wts_np,
            "nrm": nrm_np,
        }
        for c in range(NCORES)
    ]
    try:
        res = bass_utils.run_bass_kernel_spmd(nc, in_maps,
                                              core_ids=list(range(NCORES)))
    except Exception:
        # transient PJRT/compile hiccups happen occasionally; retry once
        res = bass_utils.run_bass_kernel_spmd(nc, in_maps,
                                              core_ids=list(range(NCORES)))
    global last_results
    last_results = res

    # ---- unshard: cols 0:3136 = bands A (part 0-63) / B (64-127),
    # cols 3136:6272 = bands C / D ----
    out = np.empty((B, Cout, H, W), np.float32)
    for c in range(NCORES):
        o = res.results[c]["out"]                      # [BPC, 128, OUTF] u8
        ab = o[:, :, :HALFPX].reshape(BPC, 2, Cout, NB, 4, W)
        cd = o[:, :, HALFPX:].reshape(BPC, 2, Cout, NB, 4, W)
        bands = np.stack([ab[:, 0], ab[:, 1], cd[:, 0], cd[:, 1]], axis=1)
        out[c * BPC:(c + 1) * BPC] = (
            bands.transpose(0, 2, 1, 3, 4, 5)
            .reshape(BPC, Cout, H, W).astype(np.float32))
    return out


# revision 29
# speedup vs baseline: 1.0420x; 1.0420x over previous
"""Trainium2 Bass kernel for nn_BasicBlock_1w4a_LUT (binarized 3x3 conv + LUT bucketize).

Data-parallel over batch: 8 NeuronCores x 4 images each; no cross-core
communication. Full inputs in, full output out; shard/unshard on the host.

v3: 4 concurrent PE streams via 64x64 array tiling (T0/T2/T8/T10), fp16
hi-only activations (K=64), 2-bank PSUM tiles, batched DVE.

Host prep:
  - Binarize the weights exactly as the reference does; the pow2 scale sw is
    folded into the LUT thresholds so device weights are exactly +/-1.
  - x is rounded to fp16 ("hi"). The dropped residual perturbs the conv
    output by sigma ~ 5e-3 against threshold spacing ~1, measured rel err
    ~5e-3 on the bucketized output (gate is 2e-2). PE multiplies fp16
    exactly and accumulates fp32.
  - Each 112-row image is split into 4 bands of 28 rows, one per PE tile.
    Bands A,B live in SBUF partitions 0-63 (padded rows 0..57), bands C,D
    in partitions 64-127 (padded rows 56..113); 64 channels per partition
    group, 2-row halo duplicated.
  - Per-channel affine z = y*s + b chosen so bucketize thresholds map
    tau3 -> 0 and tau5 -> 1 (frees DVE scalar slots).

Device, per image:
  - 7 quad-steps of 4-row chunks: per step, each of the 4 64x64 PE tiles
    accumulates 9 taps x 448 columns into its PSUM half-bank; the 4 streams
    run concurrently (1 col/cycle/tile at 2.4 GHz). PSUM tiles span 2 banks
    ([128, 2, 448] padded to 512) so one chunk-pair evacuates per scalar op.
  - ScalarE applies the per-channel affine out of PSUM; two custom DVE ops
    compute out = sum_k [z > tau_k] over all 7 thresholds in 2 passes over
    [128, 1792/1344] batches, writing u8.
  - PE warm-up matmuls run on the weights tile itself (no memset
    dependency) so the HAM clock gate opens (1.2 -> 2.4 GHz) while the
    first input slab lands.
"""

import numpy as np

# ---- problem constants (hardcoded per contract) ----
B, Cin, Cout, H, W = 32, 64, 64, 112, 112
NCORES = 8
BPC = B // NCORES          # images per core
HP = H + 2                 # 114 padded rows
WPAD = H + 2               # 114 padded cols
BAND = 28                  # rows per band; 4 bands per image
NB = 7                     # 4-row chunks per band
NCH = 4 * W                # 448 pixels per chunk (one PSUM bank)
SLABR = 58                 # padded rows per slab half (halo of 2 shared)
SLABF = SLABR * WPAD       # 6612 fp16 per partition per image
HALFPX = BAND * W          # 3136 pixels per band
OUTF = 2 * HALFPX          # 6272 output pixels per partition per image
NTAPS = 9
NWARM = 13                 # PE warm-up matmuls

_built = []
last_results = None


def _register_dve_ops():
    from concourse.dve_spec import (
        Spec, Src0, Src1, C0, C1, C3, One, Zero, lower,
        _spill_c3_to_src1, _has_src1,
    )
    import concourse.dve_ops as dve_ops
    from concourse.dve_ops import DveOp
    from concourse.dve_uop import DveOpSpec

    def register_op(name, spec):
        if name in dve_ops._SUB_OPCODE_FOR_NAME:
            for op in dve_ops.OPS:
                if op.name == name:
                    return op
            raise RuntimeError(name)
        row = max(dve_ops._SUB_OPCODE_FOR_NAME.values()) + 1
        assert row < 0x20, "custom-DVE opcode rows exhausted"
        shas = {}
        for ver in ("v3", "v4"):
            s = DveOpSpec(name=name, opcode=row, uops=lower(spec, ver=ver),
                          rd1_en=_has_src1(spec))
            shas[ver] = s.sha(ver)
        op = DveOp(name, spec, subdim=False, uops_sha=shas)
        dve_ops.OPS.append(op)
        dve_ops.CUSTOM_DVE_SPECS[name] = spec
        dve_ops._SUB_OPCODE_FOR_NAME[name] = row
        return op

    # u = (z>tau0) + (z>tau1) + (z>tau2);  tau2 rides C3 (spilled to in1 [P,1])
    bucket3 = register_op(
        "BUCKET3_ANT",
        Spec(
            body=_spill_c3_to_src1(((Src0 > C0) + (Src0 > C1)) + (Src0 > C3)),
            reference=lambda in0, in1, s0, s1, imm2: (
                (in0 > s0).astype(np.float32) + (in0 > s1)
                + (in0 > np.asarray(in1, np.float32).reshape(-1, 1))
            ),
        ),
    )
    # out = (z>0) + (z>1) + (z>tau4) + (z>tau6) + u
    bucket4acc = register_op(
        "BUCKET4ACC_ANT",
        Spec(
            body=(((Src0 > Zero) + (Src0 > One))
                  + ((Src0 > C0) + (Src0 > C1))) + Src1,
            reference=lambda in0, in1, s0, s1, imm2: (
                (in0 > 0).astype(np.float32) + (in0 > 1)
                + (in0 > s0) + (in0 > s1) + in1
            ),
        ),
    )
    return bucket3, bucket4acc


def _build():
    """Trace + compile the per-core Bass kernel (once per process)."""
    if _built:
        return _built[0]

    import concourse.bacc as bacc
    import concourse.mybir as mybir
    import concourse.tile as tile

    bucket3, bucket4acc = _register_dve_ops()

    f32, f16, u8 = mybir.dt.float32, mybir.dt.float16, mybir.dt.uint8
    nc = bacc.Bacc("TRN2", target_bir_lowering=False, debug=False,
                   num_devices=NCORES)

    xin_t = nc.dram_tensor("xin", [BPC, 128, SLABF], f16, kind="ExternalInput")
    wts_t = nc.dram_tensor("wts", [128, NTAPS, Cout], f16, kind="ExternalInput")
    nrm_t = nc.dram_tensor("nrm", [128, 7], f32, kind="ExternalInput")
    out_t = nc.dram_tensor("out", [BPC, 128, OUTF], u8, kind="ExternalOutput")

    with tile.TileContext(nc) as tc:
        with (
            tc.tile_pool(name="const", bufs=1) as cpool,
            tc.tile_pool(name="slab", bufs=2) as spool,
            tc.tile_pool(name="psum", bufs=4, space="PSUM") as ppool,
            tc.tile_pool(name="z", bufs=4) as zpool,
            tc.tile_pool(name="u", bufs=4) as upool,
            tc.tile_pool(name="o", bufs=2) as opool,
        ):
            # Input slab pieces matched to consumption order: quad-group g
            # only needs its own band rows — A/C rows 4g..4g+9 (low cols)
            # and B/D rows 28+4g..28+4g+9 (high cols). Ranges in fp16 cols
            # (x114 per padded row), deduplicated across groups:
            ga = [(0, 1140), (1140, 2052), (2052, 2964), (2964, 3192)]
            gb = [(3192, 4332), (4332, 5244), (5244, 6156), (6156, SLABF)]
            # weights first (small) — they gate the first real matmuls; the
            # group-0 ranges are split across both HWDGE queues so they land
            # as early as possible
            slab0 = spool.tile([128, SLABF], f16, tag="slab")
            wts = cpool.tile([128, NTAPS, Cout], f16)
            nrm = cpool.tile([128, 7], f32)
            nc.scalar.dma_start(out=wts[:], in_=wts_t.ap())
            nc.sync.dma_start(out=slab0[:, 0:570],
                              in_=xin_t.ap()[0, :, 0:570])
            nc.scalar.dma_start(out=slab0[:, 570:1140],
                                in_=xin_t.ap()[0, :, 570:1140])
            nc.sync.dma_start(out=slab0[:, 3192:3762],
                              in_=xin_t.ap()[0, :, 3192:3762])
            nc.scalar.dma_start(out=slab0[:, 3762:4332],
                                in_=xin_t.ap()[0, :, 3762:4332])
            nc.scalar.dma_start(out=nrm[:], in_=nrm_t.ap())
            for n, (lo, hi) in enumerate(
                    (ga[1], gb[1], ga[2], gb[2], ga[3], gb[3])):
                eng = nc.sync if n % 2 == 0 else nc.scalar
                eng.dma_start(out=slab0[:, lo:hi], in_=xin_t.ap()[0, :, lo:hi])

            # PE warm-up: junk matmuls on a DVE-memset tile — no DMA
            # dependency, so they issue as soon as the engines come up and
            # the HAM clock gate opens (1.2 -> 2.4 GHz) before the real
            # matmuls start. Same 64x64 tiling mode as the real matmuls
            # (mode switches drain the PE); only the two column tiles of
            # row-group 0 (row tiles must not hit the same PSUM bank
            # concurrently).
            # warm-ups run serialized on the FULL 128x128 array (no tiling)
            # so the HAM sees full-occupancy sustained busy: NWARM x 373ns
            # > the 3.4us window. The one mode-switch drain into 64x64 when
            # the real matmuls arrive costs ~0.3us.
            wu = cpool.tile([128, NCH], f16)
            nc.vector.memset(wu[:], 0.0)
            wps = ppool.tile([128, 2, NCH], f32, name="ps", tag="ps",
                             padded_shape=[128, 2, 512])
            for i in range(NWARM):
                nc.tensor.matmul(wps[:, i % 2, :], wu[:, 0:128], wu[:],
                                 start=True, stop=True)

            scale, bias = nrm[:, 0:1], nrm[:, 1:2]
            tau0, tau1, tau2 = nrm[:, 2:3], nrm[:, 3:4], nrm[:, 4:5]
            tau4, tau6 = nrm[:, 5:6], nrm[:, 6:7]

            for b in range(BPC):
                # input slab pieces in consumption order (see head comment)
                if b == 0:
                    slab = slab0
                else:
                    slab = spool.tile([128, SLABF], f16, tag="slab")
                    pieces = [ga[0], gb[0], ga[1], gb[1],
                              ga[2], gb[2], ga[3], gb[3]]
                    for n, (lo, hi) in enumerate(pieces):
                        eng = nc.sync if n % 2 == 0 else nc.scalar
                        eng.dma_start(out=slab[:, lo:hi],
                                      in_=xin_t.ap()[b, :, lo:hi])
                slabv = slab[:].rearrange("p (r w) -> p r w", w=WPAD)

                oslab = opool.tile([128, OUTF], u8)
                # DVE batches (chunk groups -> columns): small batches for
                # image 0 so the DVE chain starts as early as possible; the
                # DVE never idles once started, so later images use maximal
                # batches to minimize per-op overhead.
                if b == 0:
                    batching = (((0,), 896), ((1,), 896), ((2, 3), 1344))
                elif b == BPC - 1:
                    batching = (((0, 1), 1792), ((2,), 896), ((3,), 448))
                else:
                    batching = (((0, 1), 1792), ((2, 3), 1344))
                coff = 0
                for glist, bcols in batching:
                    batch_col = coff
                    coff += bcols
                    zA = zpool.tile([128, bcols], f32, name="zA", tag="z")
                    zB = zpool.tile([128, bcols], f32, name="zB", tag="z")
                    zoff = 0
                    for g in glist:
                        chunks = (2 * g, 2 * g + 1) if g < 3 else (6,)
                        ncols = NCH * len(chunks)
                        psA = ppool.tile([128, 2, NCH], f32, name="ps",
                                         tag="ps", padded_shape=[128, 2, 512])
                        psB = ppool.tile([128, 2, NCH], f32, name="ps",
                                         tag="ps", padded_shape=[128, 2, 512])
                        # 4 concurrent streams: T0=(0,0) band A, T2=(0,64)
                        # band B, T8=(64,0) band C, T10=(64,64) band D.
                        for t in range(NTAPS):
                            dh, dw = divmod(t, 3)
                            st, sp = (t == 0), (t == NTAPS - 1)
                            for k, c in enumerate(chunks):
                                rlo = 4 * c + dh
                                rhi = BAND + rlo
                                nc.tensor.matmul(
                                    psA[0:64, k, :], wts[0:64, t, :],
                                    slabv[0:64, rlo:rlo + 4, dw:dw + W],
                                    tile_position=(0, 0), start=st, stop=sp)
                                nc.tensor.matmul(
                                    psA[64:128, k, :], wts[0:64, t, :],
                                    slabv[0:64, rhi:rhi + 4, dw:dw + W],
                                    tile_position=(0, 64), start=st, stop=sp)
                                nc.tensor.matmul(
                                    psB[0:64, k, :], wts[64:128, t, :],
                                    slabv[64:128, rlo:rlo + 4, dw:dw + W],
                                    tile_position=(64, 0), start=st, stop=sp)
                                nc.tensor.matmul(
                                    psB[64:128, k, :], wts[64:128, t, :],
                                    slabv[64:128, rhi:rhi + 4, dw:dw + W],
                                    tile_position=(64, 64), start=st, stop=sp)
                        # evacuate the 2-bank tile with one scalar op each
                        for z, ps in ((zA, psA), (zB, psB)):
                            nc.scalar.activation(
                                z[:, zoff:zoff + ncols].rearrange(
                                    "p (k c) -> p k c", c=NCH),
                                ps[:, 0:len(chunks), :],
                                mybir.ActivationFunctionType.Identity,
                                bias=bias, scale=scale)
                        zoff += ncols

                    # 2 DVE passes per batch per band-pair, u8 into oslab
                    for z, base in ((zA, 0), (zB, HALFPX)):
                        u = upool.tile([128, bcols], mybir.dt.bfloat16)
                        nc.vector._custom_dve(
                            bucket3, out=u[:], in0=z[:],
                            in1=tau2, s0=tau0, s1=tau1)
                        col = base + batch_col
                        nc.vector._custom_dve(
                            bucket4acc, out=oslab[:, col:col + bcols],
                            in0=z[:], in1=u[:], s0=tau4, s1=tau6)

                # split output DMA so early pieces leave while later chunks
                # are still being computed; finest for the last image (tail)
                if b == BPC - 1:
                    ocuts = [0, HALFPX, HALFPX + 1792, OUTF - 896,
                             OUTF - 448, OUTF]
                else:
                    ocuts = [0, HALFPX, OUTF]
                for lo, hi in zip(ocuts[:-1], ocuts[1:]):
                    nc.sync.dma_start(out=out_t.ap()[b, :, lo:hi],
                                      in_=oslab[:, lo:hi])

    nc.compile()
    _built.append(nc)
    return nc


def _binarize_weights(w):
    """Exactly the reference's fp32 binarization. Returns (sign in {-1,0,1}, sw)."""
    w = np.asarray(w, np.float32)
    C = w.shape[0]
    wf = w.reshape(C, -1)
    bw = w - wf.mean(-1)[:, None, None, None]
    bw = bw / bw.reshape(C, -1).std(-1, ddof=1)[:, None, None, None]
    mean_abs = np.abs(bw).reshape(C, -1).mean(-1)
    sw = np.exp2(np.round(np.log2(mean_abs))).astype(np.float32)
    return np.sign(bw).astype(np.float32), sw


def kernel(x, w, lut):
    x = np.ascontiguousarray(np.asarray(x, np.float32))
    w = np.asarray(w, np.float32)
    lut = np.asarray(lut, np.float32)

    nc = _build()
    from concourse import bass_utils

    # ---- weights: binarize + fold the pow2 scale into the thresholds ----
    sgn, sw = _binarize_weights(w)                     # sgn [Cout,Cin,3,3]
    t64 = lut.astype(np.float64) / sw[:, None]         # [Cout,7] thresholds

    # lhsT per tap: wts[ci, t, co] = sgn[co, ci, dh, dw]; rows 64-127 serve
    # the row-tiled PE tiles T8/T10 (same weights, SBUF partitions 64-127)
    wts = np.empty((128, NTAPS, Cout), np.float32)
    for t in range(NTAPS):
        wts[:Cin, t, :] = sgn[:, :, t // 3, t % 3].T
    wts[Cin:] = wts[:Cin]
    wts = wts.astype(np.float16)

    # ---- normalize params: z = y*s + b with tau3 -> 0, tau5 -> 1 ----
    # s>0 always; for degenerate channels (t5 == t3) use a huge power of two
    # so [z > 1] still decides [y > t3] exactly.
    t3, t5 = t64[:, 3], t64[:, 5]
    gap = t5 - t3
    s = np.where(gap > 0, 1.0 / np.where(gap > 0, gap, 1.0), 2.0 ** 100)
    bias = -t3 * s
    taus = (t64[:, [0, 1, 2, 4, 6]] - t3[:, None]) * s[:, None]
    half = np.stack([s, bias, taus[:, 0], taus[:, 1], taus[:, 2],
                     taus[:, 3], taus[:, 4]], axis=1).astype(np.float32)
    nrm = np.empty((128, 7), np.float32)
    nrm[:Cout] = half
    nrm[Cout:] = half

    # ---- fp16 slabs: bands A,B (padded rows 0..57) in partitions 0-63,
    # bands C,D (padded rows 56..113) in partitions 64-127 ----
    hi16 = x.astype(np.float16)
    xin = np.zeros((B, 128, SLABF), np.float16)
    view = xin.reshape(B, 128, SLABR, WPAD)
    view[:, :Cin, 1:58, 1:W + 1] = hi16[:, :, 0:57, :]
    view[:, Cin:, 0:57, 1:W + 1] = hi16[:, :, 55:112, :]

    # ---- run on the 8 cores (SPMD, batch-sharded) ----
    wts_np = np.ascontiguousarray(wts)
    nrm_np = np.ascontiguousarray(nrm)
    in_maps = [
        {
            "xin": np.ascontiguousarray(xin[c * BPC:(c + 1) * BPC]),
            "wts": wts_np,
            "nrm": nrm_np,
        }
        for c in range(NCORES)
    ]
    try:
        res = bass_utils.run_bass_kernel_spmd(nc, in_maps,
                                              core_ids=list(range(NCORES)))
    except Exception:
        # transient PJRT/compile hiccups happen occasionally; retry once
        res = bass_utils.run_bass_kernel_spmd(nc, in_maps,
                                              core_ids=list(range(NCORES)))
    global last_results
    last_results = res

    # ---- unshard: cols 0:3136 = bands A (part 0-63) / B (64-127),
    # cols 3136:6272 = bands C / D ----
    out = np.empty((B, Cout, H, W), np.float32)
    for c in range(NCORES):
        o = res.results[c]["out"]                      # [BPC, 128, OUTF] u8
        ab = o[:, :, :HALFPX].reshape(BPC, 2, Cout, NB, 4, W)
        cd = o[:, :, HALFPX:].reshape(BPC, 2, Cout, NB, 4, W)
        bands = np.stack([ab[:, 0], ab[:, 1], cd[:, 0], cd[:, 1]], axis=1)
        out[c * BPC:(c + 1) * BPC] = (
            bands.transpose(0, 2, 1, 3, 4, 5)
            .reshape(BPC, Cout, H, W).astype(np.float32))
    return out
